# revision 43
# baseline (speedup 1.0000x reference)
"""Trainium2 Bass kernel: single-layer transformer encoder block.

reference:  LayerNorm -> fused QKV proj -> full softmax attention -> FC+LeakyReLU
inputs:     x [8, 2048, 512] f32 (+ LN gamma/beta, W_qkv [512,1536], W_fc [512,512], b_fc)

Sharding: pure data-parallel over batch -- each of the 8 NeuronCores gets one
batch element [S=2048, D=512]; weights are replicated, no collectives.

Algebraic restructuring (keeps results within bf16 noise of the reference;
cuts tensor-engine work ~19% vs the direct form):
  * scores: S = (xn Wq)(xn Wk)^T = xn (Wq Wk^T) xn^T. Host precomputes
    M = Wq Wk^T, so the K projection disappears and the key-side operand of
    the score matmuls is xn^T itself.
  * output: y = softmax(S) (xn Wv) Wfc + b. Host precomputes Wvf = Wv Wfc,
    so the FC layer disappears; v' = xn Wvf. Softmax rows sum to 1, so the
    bias folds into v' (v' += b) and y = softmax(S) v' exactly.
  * q-side LN-beta bias terms are constant per query -> softmax-invariant ->
    dropped exactly. For general gamma/beta (graded inputs have gamma=1,
    beta=0) they are applied to xn on the DVE instead, which keeps the
    M/Wvf folding exact.
  * PV computes O seq-major directly (E tiles as the matmul stationary), so
    no transpose-back: the epilogue is one ACT Prelu per 128-row tile with
    the per-partition scale = 1/Z (seq-major Z from 1-column ones-matmuls on
    the transposed bf16 denominator partials) and alpha = leaky slope.

Per-core pipeline (matmuls bf16 with f32 PSUM accumulation, ~172us HW):

  phase A  x and the (small) weights stream on the sync HWDGE ring in strict
           consumption order; x uses a p-outer row interleave (row p*16+t ->
           partition p, tile t) so every burst is one large contiguous DRAM
           run per partition (~2x the DMA rate of the (t p) layout) -- LN is
           per-row and keys/queries/outputs all share the permutation.
           Per 128-row tile: bn_stats/bn_aggr (tile 0 split into two feature
           halves to shorten the head), rstd = ACT Sqrt + fast DVE
           reciprocal, xn = (x-mean)*rstd in one DVE op, transpose to
           feature-major xnT via N=128 matmuls against the identity
           (DMA_TRANSPOSE blocks its issuing engine ~1.25us/tile -- tested
           and rejected), then the previous tile's v' row-tile (one-tile
           software pipeline to hide the xn->xnT latency) and per 4 tiles
           the q'T = M^T xnT chunk. Ordering edges keep the DMA-paced
           bn_stats from head-of-line blocking the DVE normalize chain.
  phase C  per 512-query chunk: S^T = xnT^T q'T into paired PSUM banks (one
           [128,1024] exp per pair -> E bf16; no max-subtraction, logits are
           O(1)); softmax denominators accumulate on DVE in bf16 (2x rate;
           the rounding averages down by sqrt(128) in Z); Z right after the
           first PV pass so the PE never waits on the DVE esum tail;
           O[s,d] = sum_t E^T[s,t] v'[t,d] with E stationary; one ACT Prelu
           (scale=1/Z) + DMA out per s-tile, the last tile split in d-halves
           to shorten the kernel tail; chunk-0 scores overlap phase A.
"""

import numpy as np
import ml_dtypes

import concourse.bass as bass
import concourse.mybir as mybir
import concourse.tile as tile
from concourse import bacc
from concourse.bass_utils import run_bass_kernel_spmd
from concourse.masks import make_identity
from concourse.tile_rust import add_dep_helper

F32 = mybir.dt.float32
BF16 = mybir.dt.bfloat16
F32R = mybir.dt.float32r
AF = mybir.ActivationFunctionType
OP = mybir.AluOpType

D = 512
ND = D // 128  # 4 feature tiles
LN_EPS = 1e-5
SLOPE = 0.01
N_CORES = 8


def build_nc(S=2048, has_g=False, has_b=False):
    NT = S // 128   # key/seq tiles
    NSC = S // 512  # query chunks
    SM_SCALE = float(D) ** -0.5

    nc = bacc.Bacc("TRN2", target_bir_lowering=False, debug=False)
    x_d = nc.dram_tensor("x", [S, D], F32, kind="ExternalInput")
    m_d = nc.dram_tensor("m", [128, ND, D], BF16, kind="ExternalInput")
    wvf_d = nc.dram_tensor("wvf", [128, ND, D], BF16, kind="ExternalInput")
    beff_d = nc.dram_tensor("beff", [1, D], F32, kind="ExternalInput")
    gmul_d = nc.dram_tensor("gmul", [1, D], F32, kind="ExternalInput") if has_g else None
    badd_d = nc.dram_tensor("badd", [1, D], F32, kind="ExternalInput") if has_b else None
    out_d = nc.dram_tensor("out", [S, D], F32, kind="ExternalOutput")

    with tile.TileContext(nc) as tc:
        with (
            tc.tile_pool(name="consts", bufs=1) as consts,
            tc.tile_pool(name="persist", bufs=1) as persist,
            tc.tile_pool(name="ln", bufs=6) as lnp,
            tc.tile_pool(name="eb", bufs=2) as ebp,
            tc.tile_pool(name="esb", bufs=2) as esb,
            tc.tile_pool(name="zb", bufs=2) as zbp,
            tc.tile_pool(name="yb", bufs=4) as ybp,
            tc.tile_pool(name="psS", bufs=2, space=bass.MemorySpace.PSUM) as psS,
            tc.tile_pool(name="psO", bufs=4, space=bass.MemorySpace.PSUM) as psO,
        ):
            # ---- constants ----
            m_sb = consts.tile([128, ND, D], BF16)
            wvf_sb = consts.tile([128, ND, D], BF16)
            beff_sb = consts.tile([128, D], F32)
            gmul_sb = consts.tile([128, D], F32) if has_g else None
            badd_sb = consts.tile([128, D], F32) if has_b else None
            ident = consts.tile([128, 128], BF16)
            make_identity(nc, ident)
            ones1b = consts.tile([128, 1], BF16)
            nc.vector.memset(ones1b, 1.0)
            eps_sb = consts.tile([128, 1], F32)
            nc.vector.memset(eps_sb, LN_EPS)
            zero_sb = consts.tile([128, 1], F32)
            nc.vector.memset(zero_sb, 0.0)

            # ---- persistent activations ----
            xnT = persist.tile([128, ND, S], BF16)   # xn^T: [d_in_tile, d_tile, s]
            qpT = persist.tile([128, ND, S], BF16)   # q'^T = (xn M)^T
            vfv = persist.tile([128, NT, D], BF16)   # v' = xn Wvf + beff: [t_in_tile, t_tile, d]
            x_tiles = persist.tile([128, NT, D], F32, name="x_tiles")

            # ---- DMA: everything on the fast sync HWDGE ring in consumption
            # order (the gpsimd software-DMA ring only moves ~86 GB/s; it is
            # left DMA-free so the Pool engine can run the LN stats) ----
            def _bcast(dst, src_d):
                ap = bass.AP(tensor=src_d.ap().tensor, offset=0,
                             ap=[[0, 128]] + src_d.ap().ap[1:])
                nc.sync.dma_start(out=dst, in_=ap)

            # p-outer interleave: partition p holds rows {p*NT + t}, so each
            # partition reads one large contiguous DRAM run per burst (the
            # (t p) layout yields 2KB descriptors and only ~170 GB/s). LN is
            # per-row, and keys/queries/outputs all use this same row
            # permutation consistently, so only the out DMA needs to match.
            x_r = x_d.rearrange("(p t) d -> p t d", p=128)
            out_r = out_d.rearrange("(p t) d -> p t d", p=128)

            def _xburst(lo, hi):
                lo = min(lo, NT)
                hi = min(hi, NT)
                if lo < hi:
                    nc.sync.dma_start(out=x_tiles[:, lo:hi, :],
                                      in_=x_r[:, lo:hi, :])

            # tile 0 as a quarter + three-quarters: bn_stats starts on the
            # first 64KB, and only two DMA issues (~0.6us each on the ring)
            # stand before the tiles-1-3 burst
            nc.sync.dma_start(out=x_tiles[:, 0:1, 0:128],
                              in_=x_r[:, 0:1, 0:128])
            nc.sync.dma_start(out=x_tiles[:, 0:1, 128:512],
                              in_=x_r[:, 0:1, 128:512])
            # tiles 1-3 BEFORE wvf: bn_stats(1..3) gate the xn chain ~2us
            # into the kernel, while v'(0) only needs wvf ~1.5us later; the
            # remaining weights slot between bursts just ahead of their
            # first consumer, bursts sized so each tile lands slightly
            # before its turn in the DVE LN chain (~2.2us/tile cadence)
            _xburst(1, 2)
            _xburst(2, 3)
            nc.sync.dma_start(out=wvf_sb, in_=wvf_d[:])
            _xburst(3, 5)
            _bcast(beff_sb, beff_d)
            if has_g:
                _bcast(gmul_sb, gmul_d)
            if has_b:
                _bcast(badd_sb, badd_d)
            _xburst(5, 8)
            nc.sync.dma_start(out=m_sb, in_=m_d[:])
            _xburst(8, 12)
            _xburst(12, NT)

            def emit_score_pairs(sc, E, esum, tp_lo, tp_hi, do_esum=True):
                # scores + exp; softmax denominators accumulate on DVE
                # (esum[p,s] = sum_tt E[tt*128+p, s]). For the chunk-0
                # overlap the adds are deferred to phase C (do_esum=False):
                # during phase A the DVE paces the xn chain, in early phase C
                # it is idle.
                for tp in range(tp_lo, tp_hi):
                    ps = psS.tile([128, 2, 512], F32, tag="mm", name="pss")
                    for half in range(2):
                        tt = 2 * tp + half
                        for et in range(ND):
                            nc.tensor.matmul(
                                ps[:, half, :],
                                xnT[:, et, tt * 128:(tt + 1) * 128],
                                qpT[:, et, sc * 512:(sc + 1) * 512],
                                start=(et == 0), stop=(et == ND - 1),
                            )
                    nc.scalar.activation(
                        out=E[:, 2 * tp:2 * tp + 2, :], in_=ps, func=AF.Exp,
                        bias=zero_sb, scale=SM_SCALE,
                    )
                    if not do_esum:
                        continue
                    if tp == tp_lo == 0:
                        nc.vector.tensor_copy(out=esum, in_=E[:, 0, :])
                        nc.vector.tensor_add(out=esum, in0=esum,
                                             in1=E[:, 1, :])
                    else:
                        for half in range(2):
                            nc.vector.tensor_add(
                                out=esum, in0=esum,
                                in1=E[:, 2 * tp + half, :])

            # ---- phase A: LN + transpose + v' + q', pipelined per tile ----
            def emit_v(j):
                # v' row-tile (emitted one tile late so the PE fills the
                # xn -> transpose -> xnT-copy latency with the next
                # transpose); the output bias rides along via the DVE add
                # (softmax rows sum to 1)
                psv = psS.tile([128, 512], F32, tag="mm", name="psv")
                for dt in range(ND):
                    nc.tensor.matmul(
                        psv,
                        xnT[:, dt, j * 128:(j + 1) * 128],
                        wvf_sb[:, dt, :],
                        start=(dt == 0), stop=(dt == ND - 1),
                    )
                nc.vector.tensor_add(out=vfv[:, j, :], in0=psv, in1=beff_sb)

            xn_insts = []
            for it in range(NT):
                if it == 0:
                    stat0 = lnp.tile([128, 2, 6], F32, tag="stat0")
                    nc.vector.bn_stats(out=stat0[:, 0, :],
                                       in_=x_tiles[:, 0, 0:128])
                    nc.vector.bn_stats(out=stat0[:, 1, :],
                                       in_=x_tiles[:, 0, 128:512])
                    stat = stat0
                else:
                    stat = lnp.tile([128, 6], F32, tag="stat")
                    bn_inst = nc.vector.bn_stats(out=stat,
                                                 in_=x_tiles[:, it, :])
                    # keep the DVE stream interleaved: without this edge the
                    # scheduler front-loads the (DMA-paced) bn_stats and the
                    # normalize chain head-of-line blocks behind them (the
                    # it==1 edge matters most: bn_stats(1) waits on the second
                    # x burst and would stall recip(0)/xn(0) by ~3us)
                    add_dep_helper(bn_inst.ins, xn_insts[max(it - 2, 0)].ins,
                                   sync=False, reason="interleave LN chain")
                mv = lnp.tile([128, 2], F32, tag="mv")
                nc.vector.bn_aggr(out=mv, in_=stat)
                stdv = lnp.tile([128, 1], F32, tag="stdv")
                rstd = lnp.tile([128, 1], F32, tag="rstd")
                xn = lnp.tile([128, D], BF16, tag="xn")
                # sqrt at NORMAL priority: boosting it sorts all (DMA-gated)
                # sqrts ahead of the ready xnT copies in the ACT stream and
                # head-of-line blocks them
                nc.scalar.activation(out=stdv, in_=mv[:, 1:2],
                                     func=AF.Sqrt, bias=eps_sb)
                # high priority: don't let later (DMA-paced) bn_stats get
                # ahead of the normalize chain in the in-order DVE stream
                with tc.high_priority():
                    nc.vector.reciprocal_approx_fast(out=rstd, in_=stdv)
                    if has_g or has_b:
                        xnf = lnp.tile([128, D], F32, tag="xnf")
                        nc.vector.tensor_scalar(
                            out=xnf, in0=x_tiles[:, it, :], scalar1=mv[:, 0:1],
                            scalar2=rstd, op0=OP.subtract, op1=OP.mult,
                        )
                        if has_g:
                            nc.vector.tensor_mul(out=xnf, in0=xnf, in1=gmul_sb)
                        if has_b:
                            xn_insts.append(nc.vector.tensor_add(
                                out=xn, in0=xnf, in1=badd_sb))
                        else:
                            xn_insts.append(nc.vector.tensor_copy(
                                out=xn, in_=xnf))
                    elif it == 0:
                        # normalize tile 0 in halves: the first two transpose
                        # matmuls only need columns 0-255, so the PE starts
                        # ~0.3us earlier on the critical head chain
                        nc.vector.tensor_scalar(
                            out=xn[:, 0:256], in0=x_tiles[:, 0, 0:256],
                            scalar1=mv[:, 0:1], scalar2=rstd,
                            op0=OP.subtract, op1=OP.mult,
                        )
                        xn_insts.append(nc.vector.tensor_scalar(
                            out=xn[:, 256:512], in0=x_tiles[:, 0, 256:512],
                            scalar1=mv[:, 0:1], scalar2=rstd,
                            op0=OP.subtract, op1=OP.mult,
                        ))
                    else:
                        xn_insts.append(nc.vector.tensor_scalar(
                            out=xn, in0=x_tiles[:, it, :], scalar1=mv[:, 0:1],
                            scalar2=rstd, op0=OP.subtract, op1=OP.mult,
                        ))
                # transpose via regular N=128 bf16 matmul against identity
                # (DMA_TRANSPOSE blocks its issuing engine ~1.25us per tile,
                # so the XBAR path loses); pt rides the phase-A-idle psO pool
                pt = psO.tile([128, ND, 128], F32, tag="o", name="pt")
                for j in range(ND):
                    nc.tensor.matmul(
                        pt[:, j, :],
                        xn[:, j * 128:(j + 1) * 128],
                        ident,
                        start=True, stop=True,
                    )
                nc.scalar.activation(
                    out=xnT[:, :, it * 128:(it + 1) * 128], in_=pt,
                    func=AF.Copy, bias=0.0,
                )
                if it >= 1:
                    emit_v(it - 1)
                if it == NT - 1:
                    emit_v(it)

                # after each group of 4 tiles, the matching q'T chunk
                if it % 4 == 3:
                    sc = it // 4
                    for et in range(ND):
                        psq = psS.tile([128, 512], F32, tag="mm", name="psq")
                        for dt in range(ND):
                            nc.tensor.matmul(
                                psq,
                                m_sb[:, dt, et * 128:(et + 1) * 128],
                                xnT[:, dt, sc * 512:(sc + 1) * 512],
                                start=(dt == 0), stop=(dt == ND - 1),
                            )
                        nc.scalar.activation(
                            out=qpT[:, et, sc * 512:(sc + 1) * 512], in_=psq,
                            func=AF.Copy, bias=0.0,
                        )
                    # overlap chunk-0 attention with the rest of phase A:
                    # its score pairs only need qpT[0] + the xnT tiles so
                    # far. Emission stays batched per 4-tile group: finer
                    # batching pulls the exps (and the 1.3us exp-table load)
                    # earlier into phase A, where they alternate with the
                    # sqrt table set and thrash the ACT stream.
                    if NSC > 1:
                        if sc == 0:
                            E0 = ebp.tile([128, NT, 512], BF16, tag="E",
                                          name="E0")
                            es0 = esb.tile([128, 512], BF16, tag="es",
                                           name="es0")
                            c0_done = 0
                        else:
                            hi = min((it + 1) // 2, NT // 2)
                            emit_score_pairs(0, E0, es0, c0_done, hi,
                                             do_esum=False)
                            c0_done = hi

            # ---- phase C: attention + epilogue, per query chunk ----
            for sc in range(NSC):
                if NSC > 1 and sc == 0:
                    E = E0
                    esum = es0
                    emit_score_pairs(0, E, esum, c0_done, NT // 2,
                                     do_esum=False)
                    nc.vector.tensor_copy(out=esum, in_=E[:, 0, :])
                    for tt in range(1, NT):
                        nc.vector.tensor_add(out=esum, in0=esum,
                                             in1=E[:, tt, :])
                else:
                    E = ebp.tile([128, NT, 512], BF16, tag="E")
                    esum = esb.tile([128, 512], BF16, tag="es", name="esum")
                    emit_score_pairs(sc, E, esum, 0, NT // 2)
                zinv = zbp.tile([128, 4], F32, tag="zinv")
                for ss in range(4):
                    if sc == NSC - 1 and ss >= 2:
                        # final output tile: accumulate in d-halves, each in
                        # its OWN psum tile (a shared tile would serialize
                        # the first Prelu behind the second half's writes),
                        # so the closing Prelu+DMA overlaps the last matmuls
                        for half in range(2):
                            oph = psO.tile([128, 256], F32, tag="o",
                                           name=f"oph{half}")
                            for tt in range(NT):
                                nc.tensor.matmul(
                                    oph,
                                    E[:, tt, ss * 128:(ss + 1) * 128],
                                    vfv[:, tt, half * 256:(half + 1) * 256],
                                    start=(tt == 0), stop=(tt == NT - 1),
                                )
                            yh = ybp.tile([128, 256], F32, tag="yh")
                            nc.scalar.activation(
                                out=yh, in_=oph,
                                func=AF.Prelu, bias=zero_sb,
                                scale=zinv[:, ss:ss + 1], alpha=SLOPE,
                            )
                            nc.sync.dma_start(
                                out=out_r[:, sc * 4 + ss,
                                          half * 256:(half + 1) * 256],
                                in_=yh)
                        continue
                    op = psO.tile([128, 512], F32, tag="o", name=f"op{ss}")
                    for tt in range(NT):
                        nc.tensor.matmul(
                            op,
                            E[:, tt, ss * 128:(ss + 1) * 128],
                            vfv[:, tt, :],
                            start=(tt == 0), stop=(tt == NT - 1),
                        )
                    if ss == 0:
                        # Z after the first PV pass: PV needs only E, so the
                        # PE isn't stalled waiting for the DVE esum tail.
                        # Seq-major Z: zp[s,0] = sum_p esum[p, s].
                        # seq-major Z via 1-col ones-matmuls on the bf16
                        # denominator partials (esum is bf16: DVE adds run at
                        # 2x rate and the per-partition rounding averages
                        # down by sqrt(128) in Z -> ~0.04% error)
                        zp = psS.tile([128, 4], F32, tag="mm", name="zp")
                        for zz in range(4):
                            nc.tensor.matmul(
                                zp[:, zz:zz + 1],
                                esum[:, zz * 128:(zz + 1) * 128],
                                ones1b,
                                start=True, stop=True,
                            )
                        nc.vector.reciprocal_approx_fast(out=zinv, in_=zp)
                    # y = Prelu(op * (1/Z)): bias already folded into v'
                    yt = ybp.tile([128, D], F32, tag="y")
                    nc.scalar.activation(
                        out=yt, in_=op, func=AF.Prelu,
                        bias=zero_sb, scale=zinv[:, ss:ss + 1], alpha=SLOPE,
                    )
                    nc.sync.dma_start(out=out_r[:, sc * 4 + ss, :], in_=yt)

    nc.compile()
    return nc


_NC_CACHE = {}


def _get_nc(S, has_g, has_b):
    key = (S, has_g, has_b)
    if key not in _NC_CACHE:
        _NC_CACHE[key] = build_nc(S, has_g, has_b)
    return _NC_CACHE[key]


def prep_inputs(x, ln_gamma, ln_beta, W_qkv, W_fc, b_fc):
    bf = ml_dtypes.bfloat16
    W_qkv = np.asarray(W_qkv, dtype=np.float32)
    W_fc = np.asarray(W_fc, dtype=np.float32)
    g = np.asarray(ln_gamma, dtype=np.float32)
    b = np.asarray(ln_beta, dtype=np.float32)
    has_g = not bool(np.all(g == 1.0))
    has_b = bool(np.any(b != 0.0))
    Wq = W_qkv[:, :D]
    Wk = W_qkv[:, D:2 * D]
    Wv = W_qkv[:, 2 * D:]
    M = Wq @ Wk.T
    Wvf = Wv @ W_fc
    m_t = np.ascontiguousarray(
        M.reshape(ND, 128, D).transpose(1, 0, 2)).astype(bf)
    wvf_t = np.ascontiguousarray(
        Wvf.reshape(ND, 128, D).transpose(1, 0, 2)).astype(bf)
    beff = np.asarray(b_fc, dtype=np.float32).reshape(1, D)
    extras = {}
    if has_g:
        extras["gmul"] = np.ascontiguousarray(g.reshape(1, D))
    if has_b:
        extras["badd"] = np.ascontiguousarray(b.reshape(1, D))
    return m_t, wvf_t, beff, extras, has_g, has_b


def run(x, ln_gamma, ln_beta, W_qkv, W_fc, b_fc, trace=False):
    x = np.asarray(x, dtype=np.float32)
    B, S, Din = x.shape
    assert B == N_CORES and Din == D and S % 512 == 0, (B, S, Din)
    m_t, wvf_t, beff, extras, has_g, has_b = prep_inputs(
        x, ln_gamma, ln_beta, W_qkv, W_fc, b_fc)
    nc = _get_nc(S, has_g, has_b)
    in_maps = [
        {
            "x": np.ascontiguousarray(x[b]),
            "m": m_t,
            "wvf": wvf_t,
            "beff": beff,
            **extras,
        }
        for b in range(B)
    ]
    res = run_bass_kernel_spmd(nc, in_maps, core_ids=list(range(B)), trace=trace)
    out = np.stack([res.results[b]["out"] for b in range(B)]).astype(np.float32)
    return out, res


def kernel(x, ln_gamma, ln_beta, W_qkv, W_fc, b_fc):
    out, _ = run(x, ln_gamma, ln_beta, W_qkv, W_fc, b_fc)
    return out


# revision 45
# speedup vs baseline: 1.1944x; 1.1944x over previous
"""Trainium2 Bass kernel: single-layer transformer encoder block.

reference:  LayerNorm -> fused QKV proj -> full softmax attention -> FC+LeakyReLU
inputs:     x [8, 2048, 512] f32 (+ LN gamma/beta, W_qkv [512,1536], W_fc [512,512], b_fc)

Sharding: pure data-parallel over batch -- each of the 8 NeuronCores gets one
batch element [S=2048, D=512]; weights are replicated, no collectives.

Algebraic restructuring (keeps results within bf16 noise of the reference;
cuts tensor-engine work ~19% vs the direct form):
  * scores: S = (xn Wq)(xn Wk)^T = xn (Wq Wk^T) xn^T. Host precomputes
    M = Wq Wk^T, so the K projection disappears and the key-side operand of
    the score matmuls is xn^T itself.
  * output: y = softmax(S) (xn Wv) Wfc + b. Host precomputes Wvf = Wv Wfc,
    so the FC layer disappears; v' = xn Wvf. Softmax rows sum to 1, so the
    bias folds into v' (v' += b) and y = softmax(S) v' exactly.
  * q-side LN-beta bias terms are constant per query -> softmax-invariant ->
    dropped exactly. For general gamma/beta (graded inputs have gamma=1,
    beta=0) they are applied to xn on the DVE instead, which keeps the
    M/Wvf folding exact.
  * PV computes O seq-major directly (E tiles as the matmul stationary), so
    no transpose-back: the epilogue is one ACT Prelu per 128-row tile with
    the per-partition scale = 1/Z (seq-major Z from 1-column ones-matmuls on
    the transposed bf16 denominator partials) and alpha = leaky slope.

Per-core pipeline (matmuls bf16 with f32 PSUM accumulation, ~172us HW):

  phase A  x and the (small) weights stream on the sync HWDGE ring in strict
           consumption order; x uses a p-outer row interleave (row p*16+t ->
           partition p, tile t) so every burst is one large contiguous DRAM
           run per partition (~2x the DMA rate of the (t p) layout) -- LN is
           per-row and keys/queries/outputs all share the permutation.
           Per 128-row tile: bn_stats/bn_aggr (tile 0 split into two feature
           halves to shorten the head), rstd = ACT Sqrt + fast DVE
           reciprocal, xn = (x-mean)*rstd in one DVE op, transpose to
           feature-major xnT via N=128 matmuls against the identity
           (DMA_TRANSPOSE blocks its issuing engine ~1.25us/tile -- tested
           and rejected), then the previous tile's v' row-tile (one-tile
           software pipeline to hide the xn->xnT latency) and per 4 tiles
           the q'T = M^T xnT chunk. Ordering edges keep the DMA-paced
           bn_stats from head-of-line blocking the DVE normalize chain.
  phase C  per 512-query chunk: S^T = xnT^T q'T into paired PSUM banks (one
           [128,1024] exp per pair -> E bf16; no max-subtraction, logits are
           O(1)); softmax denominators accumulate on DVE in bf16 (2x rate;
           the rounding averages down by sqrt(128) in Z); Z right after the
           first PV pass so the PE never waits on the DVE esum tail;
           O[s,d] = sum_t E^T[s,t] v'[t,d] with E stationary; one ACT Prelu
           (scale=1/Z) + DMA out per s-tile, the last tile split in d-halves
           to shorten the kernel tail; chunk-0 scores overlap phase A.
"""

import numpy as np
import ml_dtypes

import concourse.bass as bass
import concourse.mybir as mybir
import concourse.tile as tile
from concourse import bacc
from concourse.bass_utils import run_bass_kernel_spmd
from concourse.masks import make_identity
from concourse.tile_rust import add_dep_helper

F32 = mybir.dt.float32
BF16 = mybir.dt.bfloat16
F32R = mybir.dt.float32r
AF = mybir.ActivationFunctionType
OP = mybir.AluOpType

D = 512
ND = D // 128  # 4 feature tiles
LN_EPS = 1e-5
SLOPE = 0.01
N_CORES = 8


def build_nc(S=2048, has_g=False, has_b=False):
    NT = S // 128   # key/seq tiles
    NSC = S // 512  # query chunks
    SM_SCALE = float(D) ** -0.5

    nc = bacc.Bacc("TRN2", target_bir_lowering=False, debug=False)
    x_d = nc.dram_tensor("x", [S, D], F32, kind="ExternalInput")
    m_d = nc.dram_tensor("m", [128, ND, D], BF16, kind="ExternalInput")
    wvf_d = nc.dram_tensor("wvf", [128, ND, D], BF16, kind="ExternalInput")
    beff_d = nc.dram_tensor("beff", [1, D], F32, kind="ExternalInput")
    gmul_d = nc.dram_tensor("gmul", [1, D], F32, kind="ExternalInput") if has_g else None
    badd_d = nc.dram_tensor("badd", [1, D], F32, kind="ExternalInput") if has_b else None
    out_d = nc.dram_tensor("out", [S, D], F32, kind="ExternalOutput")

    with tile.TileContext(nc) as tc:
        with (
            tc.tile_pool(name="consts", bufs=1) as consts,
            tc.tile_pool(name="persist", bufs=1) as persist,
            tc.tile_pool(name="ln", bufs=6) as lnp,
            tc.tile_pool(name="eb", bufs=2) as ebp,
            tc.tile_pool(name="esb", bufs=2) as esb,
            tc.tile_pool(name="zb", bufs=2) as zbp,
            tc.tile_pool(name="yb", bufs=4) as ybp,
            tc.tile_pool(name="psS", bufs=2, space=bass.MemorySpace.PSUM) as psS,
            tc.tile_pool(name="psO", bufs=4, space=bass.MemorySpace.PSUM) as psO,
        ):
            # ---- constants ----
            m_sb = consts.tile([128, ND, D], BF16)
            wvf_sb = consts.tile([128, ND, D], BF16)
            beff_sb = consts.tile([128, D], F32)
            gmul_sb = consts.tile([128, D], F32) if has_g else None
            badd_sb = consts.tile([128, D], F32) if has_b else None
            ident = consts.tile([128, 128], BF16)
            make_identity(nc, ident)
            ones1b = consts.tile([128, 1], BF16)
            nc.vector.memset(ones1b, 1.0)
            eps_sb = consts.tile([128, 1], F32)
            nc.vector.memset(eps_sb, LN_EPS)
            zero_sb = consts.tile([128, 1], F32)
            nc.vector.memset(zero_sb, 0.0)

            # ---- persistent activations ----
            xnT = persist.tile([128, ND, S], BF16)   # xn^T: [d_in_tile, d_tile, s]
            qpT = persist.tile([128, ND, S], BF16)   # q'^T = (xn M)^T
            vfv = persist.tile([128, NT, D], BF16)   # v' = xn Wvf + beff: [t_in_tile, t_tile, d]
            x_tiles = persist.tile([128, NT, D], F32, name="x_tiles")

            # ---- DMA: everything on the fast sync HWDGE ring in consumption
            # order (the gpsimd software-DMA ring only moves ~86 GB/s; it is
            # left DMA-free so the Pool engine can run the LN stats) ----
            def _bcast(dst, src_d):
                ap = bass.AP(tensor=src_d.ap().tensor, offset=0,
                             ap=[[0, 128]] + src_d.ap().ap[1:])
                nc.sync.dma_start(out=dst, in_=ap)

            # p-outer interleave: partition p holds rows {p*NT + t}, so each
            # partition reads one large contiguous DRAM run per burst (the
            # (t p) layout yields 2KB descriptors and only ~170 GB/s). LN is
            # per-row, and keys/queries/outputs all use this same row
            # permutation consistently, so only the out DMA needs to match.
            x_r = x_d.rearrange("(p t) d -> p t d", p=128)
            out_r = out_d.rearrange("(p t) d -> p t d", p=128)

            def _xburst(lo, hi):
                lo = min(lo, NT)
                hi = min(hi, NT)
                if lo < hi:
                    nc.sync.dma_start(out=x_tiles[:, lo:hi, :],
                                      in_=x_r[:, lo:hi, :])

            # tile 0 as a quarter + three-quarters: bn_stats starts on the
            # first 64KB, and only two DMA issues (~0.6us each on the ring)
            # stand before the tiles-1-3 burst
            nc.sync.dma_start(out=x_tiles[:, 0:1, 0:128],
                              in_=x_r[:, 0:1, 0:128])
            nc.sync.dma_start(out=x_tiles[:, 0:1, 128:512],
                              in_=x_r[:, 0:1, 128:512])
            # tiles 1-3 BEFORE wvf: bn_stats(1..3) gate the xn chain ~2us
            # into the kernel, while v'(0) only needs wvf ~1.5us later; the
            # remaining weights slot between bursts just ahead of their
            # first consumer, bursts sized so each tile lands slightly
            # before its turn in the DVE LN chain (~2.2us/tile cadence)
            _xburst(1, 2)
            _xburst(2, 3)
            nc.sync.dma_start(out=wvf_sb, in_=wvf_d[:])
            _xburst(3, 5)
            _bcast(beff_sb, beff_d)
            if has_g:
                _bcast(gmul_sb, gmul_d)
            if has_b:
                _bcast(badd_sb, badd_d)
            _xburst(5, 8)
            nc.sync.dma_start(out=m_sb, in_=m_d[:])
            _xburst(8, 12)
            _xburst(12, NT)

            def emit_score_pairs(sc, E, esum, tp_lo, tp_hi, do_esum=True):
                # scores + exp; softmax denominators accumulate on DVE
                # (esum[p,s] = sum_tt E[tt*128+p, s]). For the chunk-0
                # overlap the adds are deferred to phase C (do_esum=False):
                # during phase A the DVE paces the xn chain, in early phase C
                # it is idle.
                for tp in range(tp_lo, tp_hi):
                    ps = psS.tile([128, 2, 512], F32, tag="mm", name="pss")
                    for half in range(2):
                        tt = 2 * tp + half
                        for et in range(ND):
                            nc.tensor.matmul(
                                ps[:, half, :],
                                xnT[:, et, tt * 128:(tt + 1) * 128],
                                qpT[:, et, sc * 512:(sc + 1) * 512],
                                start=(et == 0), stop=(et == ND - 1),
                            )
                    nc.scalar.activation(
                        out=E[:, 2 * tp:2 * tp + 2, :], in_=ps, func=AF.Exp,
                        bias=zero_sb, scale=SM_SCALE,
                    )
                    if not do_esum:
                        continue
                    if tp == tp_lo == 0:
                        nc.vector.tensor_copy(out=esum, in_=E[:, 0, :])
                        nc.vector.tensor_add(out=esum, in0=esum,
                                             in1=E[:, 1, :])
                    else:
                        for half in range(2):
                            nc.vector.tensor_add(
                                out=esum, in0=esum,
                                in1=E[:, 2 * tp + half, :])

            # ---- phase A: LN + transpose + v' + q', pipelined per tile ----
            def emit_v(j):
                # v' row-tile (emitted one tile late so the PE fills the
                # xn -> transpose -> xnT-copy latency with the next
                # transpose); the output bias rides along via the DVE add
                # (softmax rows sum to 1)
                psv = psS.tile([128, 512], F32, tag="mm", name="psv")
                for dt in range(ND):
                    nc.tensor.matmul(
                        psv,
                        xnT[:, dt, j * 128:(j + 1) * 128],
                        wvf_sb[:, dt, :],
                        start=(dt == 0), stop=(dt == ND - 1),
                    )
                nc.vector.tensor_add(out=vfv[:, j, :], in0=psv, in1=beff_sb)

            xn_insts = []
            for it in range(NT):
                if it == 0:
                    stat0 = lnp.tile([128, 2, 6], F32, tag="stat0")
                    nc.vector.bn_stats(out=stat0[:, 0, :],
                                       in_=x_tiles[:, 0, 0:128])
                    nc.vector.bn_stats(out=stat0[:, 1, :],
                                       in_=x_tiles[:, 0, 128:512])
                    stat = stat0
                else:
                    stat = lnp.tile([128, 6], F32, tag="stat")
                    bn_inst = nc.vector.bn_stats(out=stat,
                                                 in_=x_tiles[:, it, :])
                    # keep the DVE stream interleaved: without this edge the
                    # scheduler front-loads the (DMA-paced) bn_stats and the
                    # normalize chain head-of-line blocks behind them (the
                    # it==1 edge matters most: bn_stats(1) waits on the second
                    # x burst and would stall recip(0)/xn(0) by ~3us)
                    add_dep_helper(bn_inst.ins, xn_insts[max(it - 2, 0)].ins,
                                   sync=False, reason="interleave LN chain")
                mv = lnp.tile([128, 2], F32, tag="mv")
                nc.vector.bn_aggr(out=mv, in_=stat)
                stdv = lnp.tile([128, 1], F32, tag="stdv")
                rstd = lnp.tile([128, 1], F32, tag="rstd")
                xn = lnp.tile([128, D], BF16, tag="xn")
                # sqrt at NORMAL priority: boosting it sorts all (DMA-gated)
                # sqrts ahead of the ready xnT copies in the ACT stream and
                # head-of-line blocks them
                nc.scalar.activation(out=stdv, in_=mv[:, 1:2],
                                     func=AF.Sqrt, bias=eps_sb)
                # high priority: don't let later (DMA-paced) bn_stats get
                # ahead of the normalize chain in the in-order DVE stream
                with tc.high_priority():
                    nc.vector.reciprocal_approx_fast(out=rstd, in_=stdv)
                    if has_g or has_b:
                        xnf = lnp.tile([128, D], F32, tag="xnf")
                        nc.vector.tensor_scalar(
                            out=xnf, in0=x_tiles[:, it, :], scalar1=mv[:, 0:1],
                            scalar2=rstd, op0=OP.subtract, op1=OP.mult,
                        )
                        if has_g:
                            nc.vector.tensor_mul(out=xnf, in0=xnf, in1=gmul_sb)
                        if has_b:
                            xn_insts.append(nc.vector.tensor_add(
                                out=xn, in0=xnf, in1=badd_sb))
                        else:
                            xn_insts.append(nc.vector.tensor_copy(
                                out=xn, in_=xnf))
                    elif it == 0:
                        # normalize tile 0 in halves: the first two transpose
                        # matmuls only need columns 0-255, so the PE starts
                        # ~0.3us earlier on the critical head chain
                        nc.vector.tensor_scalar(
                            out=xn[:, 0:256], in0=x_tiles[:, 0, 0:256],
                            scalar1=mv[:, 0:1], scalar2=rstd,
                            op0=OP.subtract, op1=OP.mult,
                        )
                        xn_insts.append(nc.vector.tensor_scalar(
                            out=xn[:, 256:512], in0=x_tiles[:, 0, 256:512],
                            scalar1=mv[:, 0:1], scalar2=rstd,
                            op0=OP.subtract, op1=OP.mult,
                        ))
                    else:
                        xn_insts.append(nc.vector.tensor_scalar(
                            out=xn, in0=x_tiles[:, it, :], scalar1=mv[:, 0:1],
                            scalar2=rstd, op0=OP.subtract, op1=OP.mult,
                        ))
                # transpose via regular N=128 bf16 matmul against identity
                # (DMA_TRANSPOSE blocks its issuing engine ~1.25us per tile,
                # so the XBAR path loses); pt rides the phase-A-idle psO pool
                pt = psO.tile([128, ND, 128], F32, tag="o", name="pt")
                for j in range(ND):
                    nc.tensor.matmul(
                        pt[:, j, :],
                        xn[:, j * 128:(j + 1) * 128],
                        ident,
                        start=True, stop=True,
                    )
                nc.scalar.activation(
                    out=xnT[:, :, it * 128:(it + 1) * 128], in_=pt,
                    func=AF.Copy, bias=0.0,
                )
                if it >= 1:
                    emit_v(it - 1)
                if it == NT - 1:
                    emit_v(it)

                # after each group of 4 tiles, the matching q'T chunk
                if it % 4 == 3:
                    sc = it // 4
                    for et in range(ND):
                        psq = psS.tile([128, 512], F32, tag="mm", name="psq")
                        for dt in range(ND):
                            nc.tensor.matmul(
                                psq,
                                m_sb[:, dt, et * 128:(et + 1) * 128],
                                xnT[:, dt, sc * 512:(sc + 1) * 512],
                                start=(dt == 0), stop=(dt == ND - 1),
                            )
                        nc.scalar.activation(
                            out=qpT[:, et, sc * 512:(sc + 1) * 512], in_=psq,
                            func=AF.Copy, bias=0.0,
                        )
                    # overlap chunk-0 attention with the rest of phase A:
                    # its score pairs only need qpT[0] + the xnT tiles so
                    # far. Emission stays batched per 4-tile group: finer
                    # batching pulls the exps (and the 1.3us exp-table load)
                    # earlier into phase A, where they alternate with the
                    # sqrt table set and thrash the ACT stream.
                    if NSC > 1:
                        if sc == 0:
                            E0 = ebp.tile([128, NT, 512], BF16, tag="E",
                                          name="E0")
                            es0 = esb.tile([128, 512], BF16, tag="es",
                                           name="es0")
                            c0_done = 0
                        else:
                            hi = min((it + 1) // 2, NT // 2)
                            emit_score_pairs(0, E0, es0, c0_done, hi,
                                             do_esum=False)
                            c0_done = hi

            # ---- phase C: attention + epilogue, per query chunk ----
            for sc in range(NSC):
                if NSC > 1 and sc == 0:
                    E = E0
                    esum = es0
                    emit_score_pairs(0, E, esum, c0_done, NT // 2,
                                     do_esum=False)
                    nc.vector.tensor_copy(out=esum, in_=E[:, 0, :])
                    for tt in range(1, NT):
                        nc.vector.tensor_add(out=esum, in0=esum,
                                             in1=E[:, tt, :])
                else:
                    E = ebp.tile([128, NT, 512], BF16, tag="E")
                    esum = esb.tile([128, 512], BF16, tag="es", name="esum")
                    emit_score_pairs(sc, E, esum, 0, NT // 2)
                zinv = zbp.tile([128, 4], F32, tag="zinv")
                for ss in range(4):
                    if sc == NSC - 1 and ss >= 2:
                        # final output tile: accumulate in d-halves, each in
                        # its OWN psum tile (a shared tile would serialize
                        # the first Prelu behind the second half's writes),
                        # so the closing Prelu+DMA overlaps the last matmuls
                        for half in range(2):
                            oph = psO.tile([128, 256], F32, tag="o",
                                           name=f"oph{half}")
                            for tt in range(NT):
                                nc.tensor.matmul(
                                    oph,
                                    E[:, tt, ss * 128:(ss + 1) * 128],
                                    vfv[:, tt, half * 256:(half + 1) * 256],
                                    start=(tt == 0), stop=(tt == NT - 1),
                                )
                            yh = ybp.tile([128, 256], F32, tag="yh")
                            nc.scalar.activation(
                                out=yh, in_=oph,
                                func=AF.Prelu, bias=zero_sb,
                                scale=zinv[:, ss:ss + 1], alpha=SLOPE,
                            )
                            nc.sync.dma_start(
                                out=out_r[:, sc * 4 + ss,
                                          half * 256:(half + 1) * 256],
                                in_=yh)
                        continue
                    op = psO.tile([128, 512], F32, tag="o", name=f"op{ss}")
                    for tt in range(NT):
                        nc.tensor.matmul(
                            op,
                            E[:, tt, ss * 128:(ss + 1) * 128],
                            vfv[:, tt, :],
                            start=(tt == 0), stop=(tt == NT - 1),
                        )
                    if ss == 0:
                        # Z after the first PV pass: PV needs only E, so the
                        # PE isn't stalled waiting for the DVE esum tail.
                        # Seq-major Z: zp[s,0] = sum_p esum[p, s].
                        # seq-major Z via 1-col ones-matmuls on the bf16
                        # denominator partials (esum is bf16: DVE adds run at
                        # 2x rate and the per-partition rounding averages
                        # down by sqrt(128) in Z -> ~0.04% error)
                        zp = psS.tile([128, 4], F32, tag="mm", name="zp")
                        for zz in range(4):
                            nc.tensor.matmul(
                                zp[:, zz:zz + 1],
                                esum[:, zz * 128:(zz + 1) * 128],
                                ones1b,
                                start=True, stop=True,
                            )
                        nc.vector.reciprocal_approx_fast(out=zinv, in_=zp)
                    # y = Prelu(op * (1/Z)): bias already folded into v'
                    yt = ybp.tile([128, D], F32, tag="y")
                    nc.scalar.activation(
                        out=yt, in_=op, func=AF.Prelu,
                        bias=zero_sb, scale=zinv[:, ss:ss + 1], alpha=SLOPE,
                    )
                    nc.sync.dma_start(out=out_r[:, sc * 4 + ss, :], in_=yt)

    nc.compile()
    return nc


_NC_CACHE = {}


def _get_nc(S, has_g, has_b):
    key = (S, has_g, has_b)
    if key not in _NC_CACHE:
        _NC_CACHE[key] = build_nc(S, has_g, has_b)
    return _NC_CACHE[key]


def prep_inputs(x, ln_gamma, ln_beta, W_qkv, W_fc, b_fc):
    bf = ml_dtypes.bfloat16
    W_qkv = np.asarray(W_qkv, dtype=np.float32)
    W_fc = np.asarray(W_fc, dtype=np.float32)
    g = np.asarray(ln_gamma, dtype=np.float32)
    b = np.asarray(ln_beta, dtype=np.float32)
    has_g = not bool(np.all(g == 1.0))
    has_b = bool(np.any(b != 0.0))
    Wq = W_qkv[:, :D]
    Wk = W_qkv[:, D:2 * D]
    Wv = W_qkv[:, 2 * D:]
    M = Wq @ Wk.T
    Wvf = Wv @ W_fc
    m_t = np.ascontiguousarray(
        M.reshape(ND, 128, D).transpose(1, 0, 2)).astype(bf)
    wvf_t = np.ascontiguousarray(
        Wvf.reshape(ND, 128, D).transpose(1, 0, 2)).astype(bf)
    beff = np.asarray(b_fc, dtype=np.float32).reshape(1, D)
    extras = {}
    if has_g:
        extras["gmul"] = np.ascontiguousarray(g.reshape(1, D))
    if has_b:
        extras["badd"] = np.ascontiguousarray(b.reshape(1, D))
    return m_t, wvf_t, beff, extras, has_g, has_b


def run(x, ln_gamma, ln_beta, W_qkv, W_fc, b_fc, trace=False):
    x = np.asarray(x, dtype=np.float32)
    B, S, Din = x.shape
    assert B == N_CORES and Din == D and S % 512 == 0, (B, S, Din)
    m_t, wvf_t, beff, extras, has_g, has_b = prep_inputs(
        x, ln_gamma, ln_beta, W_qkv, W_fc, b_fc)
    nc = _get_nc(S, has_g, has_b)
    in_maps = [
        {
            "x": np.ascontiguousarray(x[b]),
            "m": m_t,
            "wvf": wvf_t,
            "beff": beff,
            **extras,
        }
        for b in range(B)
    ]
    res = run_bass_kernel_spmd(nc, in_maps, core_ids=list(range(B)), trace=trace)
    out = np.stack([res.results[b]["out"] for b in range(B)]).astype(np.float32)
    return out, res


def kernel(x, ln_gamma, ln_beta, W_qkv, W_fc, b_fc):
    out, _ = run(x, ln_gamma, ln_beta, W_qkv, W_fc, b_fc)
    return out
